# revision 15
# baseline (speedup 1.0000x reference)
"""Trainium2 Bass kernel for nn_CompressK (segment_reduce).

Computes, per sequence, a mean over sliding windows of KERNEL_SIZE=32 rows
at stride KERNEL_STRIDE=16 of k (viewed as (rows, head_num_k*head_dim)),
returning (compressed_k, cu_comp) exactly like the reference.

Hardware strategy (8 NeuronCores):
  - 4 sequences of 16384 rows -> 1023 chunks each. Two cores per sequence:
    core 2s   computes chunks   0..511 (rows [0,     8208) of seq s)
    core 2s+1 computes chunks 511..1022 (rows [8176, 16384) of seq s)
    Both produce 512 chunks; the duplicated chunk 511 is dropped on gather.
  - Sliding mean = block sums + adjacent add: with 16-row blocks,
    out[c] = (B[c] + B[c+1]) / 32. Host pre-blocks the shard so block p of
    tile i sits on SBUF partition p: kb[i, p, j*F+f] = row (2048i+128p+j).
  - Per tile the VectorE accumulates the 16 rows with in-place fp32 adds
    (exact, no PE weight-load serialization, no PSUM); the B[c+1] term
    comes from an SBUF->SBUF partition-shift DMA, with the cross-tile
    boundary block sums computed on host (tiny) to avoid serialization.
    ScalarE applies the 1/32 scale on the way to the output tile.
"""

import numpy as np

KERNEL_SIZE = 32
KERNEL_STRIDE = 16
HEAD_NUM_K = 4
HEAD_DIM = 128
BATCH = 4
SEQ_LEN = 16384
F = HEAD_NUM_K * HEAD_DIM          # 512 features per row
N_CORES = 8
CHUNKS_PER_SEQ = (SEQ_LEN - KERNEL_SIZE) // KERNEL_STRIDE + 1  # 1023
CHUNKS_PER_CORE = 512
TILES = 4                          # 128-block tiles per core
TILE_ROWS = 128 * KERNEL_STRIDE    # 2048 rows per tile
ROWS_PER_CORE = TILES * TILE_ROWS + KERNEL_STRIDE  # 8208
JPB = KERNEL_STRIDE                # rows per block (16)

_CACHE = {}


def _build_module():
    import concourse.tile as tile
    from concourse import bacc, mybir

    nc = bacc.Bacc("TRN2", target_bir_lowering=False, debug=False)
    f32 = mybir.dt.float32
    # kb[i, p, j*F+f] = k_shard[2048*i + 128*p + j*? ...] -- host blocks rows
    # so partition p of tile i holds block (128i+p)'s 16 rows contiguously.
    kb = nc.dram_tensor("kb", [TILES, 128, JPB * F], f32,
                        kind="ExternalInput").ap()
    # bnd[i] = fp32 block sum of block 128*(i+1) (the first block of the next
    # tile; for i=3 the 16 rows past the last tile) -- computed on host.
    bnd = nc.dram_tensor("bnd", [TILES, 1, F], f32, kind="ExternalInput").ap()
    out = nc.dram_tensor("out", [CHUNKS_PER_CORE, F], f32,
                         kind="ExternalOutput").ap()
    HALF = JPB * F // 2            # 4096 floats: rows j=0..7

    with tile.TileContext(nc) as tc:
        with tc.tile_pool(name="data", bufs=3) as dpool, \
             tc.tile_pool(name="shp", bufs=2) as spool, \
             tc.tile_pool(name="outp", bufs=2) as opool:
            for i in range(TILES):
                d = dpool.tile([128, JPB * F], f32, tag="d", name=f"d{i}")
                nc.sync.dma_start(d[:, 0:HALF], kb[i, :, 0:HALF])
                nc.sync.dma_start(d[:, HALF:], kb[i, :, HALF:])
                # Block sums, accumulated in place into columns [0:F) /
                # [HALF:HALF+F) so each half only depends on its own DMA.
                for j in range(1, JPB // 2):
                    nc.vector.tensor_add(d[:, 0:F], d[:, 0:F],
                                         d[:, F * j: F * (j + 1)])
                    nc.vector.tensor_add(d[:, HALF: HALF + F],
                                         d[:, HALF: HALF + F],
                                         d[:, HALF + F * j: HALF + F * (j + 1)])
                nc.vector.tensor_add(d[:, 0:F], d[:, 0:F], d[:, HALF: HALF + F])

                # sh[p] = B[128i + p + 1]: partition-shift of the block sums,
                # boundary block from the host-computed bnd input.
                sh = spool.tile([128, F], f32, tag="sh", name=f"sh{i}")
                nc.scalar.dma_start(sh[0:127, :], d[1:128, 0:F])
                nc.scalar.dma_start(sh[127:128, :], bnd[i])
                nc.vector.tensor_add(sh[:], sh[:], d[:, 0:F])

                ot = opool.tile([128, F], f32, tag="ot", name=f"ot{i}")
                nc.scalar.mul(ot[:], sh[:], 1.0 / KERNEL_SIZE)
                nc.scalar.dma_start(out[128 * i: 128 * (i + 1), :], ot[:])
    nc.compile()
    return nc


def _get_module():
    if "nc" not in _CACHE:
        _CACHE["nc"] = _build_module()
    return _CACHE["nc"]


def _calc_chunks_with_stride(cu_seqlens_np, chunk_size, stride):
    """Host-side mirror of the reference index computation."""
    cu = np.asarray(cu_seqlens_np, dtype=np.int64)
    batch_sizes = cu[1:] - cu[:-1]
    max_seq_len = int(batch_sizes.max())
    max_chunks = max((max_seq_len - chunk_size) // stride + 1, 0)
    offsets = np.arange(0, max_chunks * stride, stride, dtype=np.int64)
    seq_starts = cu[:-1]
    chunk_start = seq_starts[:, None] + offsets[None, :]
    chunk_end = chunk_start + chunk_size
    valid = chunk_end <= (seq_starts[:, None] + batch_sizes[:, None])
    valid_starts = chunk_start[valid]
    inner = np.arange(chunk_size, dtype=np.int64)[None, :]
    flat_idx = (valid_starts[:, None] + inner).reshape(-1)
    n_per_batch = valid.sum(axis=1)
    cu_comp = np.zeros(len(cu), dtype=np.int32)
    cu_comp[1:] = np.cumsum(n_per_batch)
    return flat_idx, cu_comp


def _numpy_fallback(k, cu_seqlens):
    flat_idx, cu_comp = _calc_chunks_with_stride(
        np.asarray(cu_seqlens), KERNEL_SIZE, KERNEL_STRIDE)
    k = np.asarray(k)
    gathered = k[flat_idx].reshape(-1, KERNEL_SIZE, k.shape[1], k.shape[2])
    return gathered.mean(axis=1, dtype=np.float64).astype(k.dtype), cu_comp


def _prep_shard(x: np.ndarray):
    """x: (ROWS_PER_CORE, F) fp32 -> (kb (TILES,128,16*F), bnd (TILES,1,F))."""
    kb = x[:TILES * TILE_ROWS].reshape(TILES, 128, JPB, F) \
        .reshape(TILES, 128, JPB * F)
    bnd = np.stack([
        x[TILE_ROWS * (i + 1): TILE_ROWS * (i + 1) + JPB].sum(
            axis=0, dtype=np.float32, keepdims=True)
        for i in range(TILES)])
    return np.ascontiguousarray(kb), np.ascontiguousarray(bnd)


def _run_hw(k2: np.ndarray, trace: bool = False, **spmd_kwargs):
    """k2: (BATCH*SEQ_LEN, F) fp32 contiguous. Returns (per-core outs, results)."""
    from concourse.bass_utils import run_bass_kernel_spmd

    nc = _get_module()
    in_maps = []
    for s in range(BATCH):
        for r0 in (s * SEQ_LEN, s * SEQ_LEN + SEQ_LEN - ROWS_PER_CORE):
            kb, bnd = _prep_shard(k2[r0: r0 + ROWS_PER_CORE])
            in_maps.append({"kb": kb, "bnd": bnd})
    res = run_bass_kernel_spmd(nc, in_maps, core_ids=list(range(N_CORES)),
                               trace=trace, **spmd_kwargs)
    outs = [res.results[i]["out"] for i in range(N_CORES)]
    return outs, res


def _assemble(outs) -> np.ndarray:
    seqs = []
    for s in range(BATCH):
        a = outs[2 * s]          # chunks 0..511
        b = outs[2 * s + 1]      # chunks 511..1022 (first is dup of a[511])
        seqs.append(np.concatenate([a, b[1:]], axis=0))
    comp = np.concatenate(seqs, axis=0)
    return np.ascontiguousarray(comp.reshape(-1, HEAD_NUM_K, HEAD_DIM))


def kernel(k, cu_seqlens):
    k = np.asarray(k)
    cu_seqlens = np.asarray(cu_seqlens)
    expected_cu = np.arange(BATCH + 1, dtype=np.int64) * SEQ_LEN
    if (k.shape != (BATCH * SEQ_LEN, HEAD_NUM_K, HEAD_DIM)
            or k.dtype != np.float32
            or cu_seqlens.shape != (BATCH + 1,)
            or not np.array_equal(np.asarray(cu_seqlens, np.int64), expected_cu)):
        return _numpy_fallback(k, cu_seqlens)

    _, cu_comp = _calc_chunks_with_stride(cu_seqlens, KERNEL_SIZE, KERNEL_STRIDE)
    k2 = np.ascontiguousarray(k.reshape(BATCH * SEQ_LEN, F))
    outs, _ = _run_hw(k2)
    return _assemble(outs), cu_comp


# revision 16
# speedup vs baseline: 1.7945x; 1.7945x over previous
"""Trainium2 Bass kernel for nn_CompressK (segment_reduce).

Computes, per sequence, a mean over sliding windows of KERNEL_SIZE=32 rows
at stride KERNEL_STRIDE=16 of k (viewed as (rows, head_num_k*head_dim)),
returning (compressed_k, cu_comp) exactly like the reference.

Hardware strategy (8 NeuronCores):
  - 4 sequences of 16384 rows -> 1023 chunks each. Two cores per sequence:
    core 2s   computes chunks   0..511 (rows [0,     8208) of seq s)
    core 2s+1 computes chunks 511..1022 (rows [8176, 16384) of seq s)
    Both produce 512 chunks; the duplicated chunk 511 is dropped on gather.
  - Sliding mean = block sums + adjacent add: with 16-row blocks,
    out[c] = (B[c] + B[c+1]) / 32. Host pre-blocks the shard so block p of
    tile i sits on SBUF partition p: kb[i, p, j*F+f] = row (2048i+128p+j).
  - Per tile the VectorE accumulates the 16 rows with in-place fp32 adds
    (exact, no PE weight-load serialization, no PSUM); the B[c+1] term
    comes from an SBUF->SBUF partition-shift DMA, with the cross-tile
    boundary block sums computed on host (tiny) to avoid serialization.
    ScalarE applies the 1/32 scale on the way to the output tile.
"""

import numpy as np

KERNEL_SIZE = 32
KERNEL_STRIDE = 16
HEAD_NUM_K = 4
HEAD_DIM = 128
BATCH = 4
SEQ_LEN = 16384
F = HEAD_NUM_K * HEAD_DIM          # 512 features per row
N_CORES = 8
CHUNKS_PER_SEQ = (SEQ_LEN - KERNEL_SIZE) // KERNEL_STRIDE + 1  # 1023
CHUNKS_PER_CORE = 512
TILES = 4                          # 128-block tiles per core
TILE_ROWS = 128 * KERNEL_STRIDE    # 2048 rows per tile
ROWS_PER_CORE = TILES * TILE_ROWS + KERNEL_STRIDE  # 8208
JPB = KERNEL_STRIDE                # rows per block (16)

_CACHE = {}


def _build_module():
    import concourse.tile as tile
    from concourse import bacc, mybir

    nc = bacc.Bacc("TRN2", target_bir_lowering=False, debug=False)
    f32 = mybir.dt.float32
    # kb[i, p, j*F+f] = k_shard[2048*i + 128*p + j*? ...] -- host blocks rows
    # so partition p of tile i holds block (128i+p)'s 16 rows contiguously.
    kb = nc.dram_tensor("kb", [TILES, 128, JPB * F], f32,
                        kind="ExternalInput").ap()
    # w2[p, m] = 1/32 iff p in (m, m+1): adjacent-block add + mean scale as
    # a single small fp32 matmul per tile (out[m,f] = sum_p w2[p,m] blk[p,f]).
    # The cross-tile boundary term (chunk 127 of each tile needs the next
    # tile's first block) is added on HOST -- it already computes those 16-row
    # sums, and the correction touches only 4 output rows per core.
    w2 = nc.dram_tensor("w2", [128, 128], f32, kind="ExternalInput").ap()
    out = nc.dram_tensor("out", [CHUNKS_PER_CORE, F], f32,
                         kind="ExternalOutput").ap()
    HALF = JPB * F // 2            # 4096 floats: rows j=0..7

    with tile.TileContext(nc) as tc:
        with tc.tile_pool(name="data", bufs=3) as dpool, \
             tc.tile_pool(name="wp", bufs=1) as wpool, \
             tc.tile_pool(name="psum", bufs=2, space="PSUM") as ppool, \
             tc.tile_pool(name="outp", bufs=2) as opool:
            w2sb = wpool.tile([128, 128], f32)
            nc.sync.dma_start(w2sb[:], w2[:])
            for i in range(TILES):
                d = dpool.tile([128, JPB * F], f32, tag="d", name=f"d{i}")
                nc.sync.dma_start(d[:, 0:HALF], kb[i, :, 0:HALF])
                nc.sync.dma_start(d[:, HALF:], kb[i, :, HALF:])
                # Block sums, accumulated in place into columns [0:F) /
                # [HALF:HALF+F) so each half only depends on its own DMA.
                for j in range(1, JPB // 2):
                    nc.vector.tensor_add(d[:, 0:F], d[:, 0:F],
                                         d[:, F * j: F * (j + 1)])
                    nc.vector.tensor_add(d[:, HALF: HALF + F],
                                         d[:, HALF: HALF + F],
                                         d[:, HALF + F * j: HALF + F * (j + 1)])
                nc.vector.tensor_add(d[:, 0:F], d[:, 0:F], d[:, HALF: HALF + F])

                ps = ppool.tile([128, F], f32, tag="ps", name=f"ps{i}")
                nc.tensor.matmul(ps[:], lhsT=w2sb[:], rhs=d[:, 0:F],
                                 start=True, stop=True)
                ot = opool.tile([128, F], f32, tag="ot", name=f"ot{i}")
                nc.scalar.copy(ot[:], ps[:])
                nc.scalar.dma_start(out[128 * i: 128 * (i + 1), :], ot[:])
    nc.compile()
    return nc


def _get_module():
    if "nc" not in _CACHE:
        _CACHE["nc"] = _build_module()
    return _CACHE["nc"]


def _calc_chunks_with_stride(cu_seqlens_np, chunk_size, stride):
    """Host-side mirror of the reference index computation."""
    cu = np.asarray(cu_seqlens_np, dtype=np.int64)
    batch_sizes = cu[1:] - cu[:-1]
    max_seq_len = int(batch_sizes.max())
    max_chunks = max((max_seq_len - chunk_size) // stride + 1, 0)
    offsets = np.arange(0, max_chunks * stride, stride, dtype=np.int64)
    seq_starts = cu[:-1]
    chunk_start = seq_starts[:, None] + offsets[None, :]
    chunk_end = chunk_start + chunk_size
    valid = chunk_end <= (seq_starts[:, None] + batch_sizes[:, None])
    valid_starts = chunk_start[valid]
    inner = np.arange(chunk_size, dtype=np.int64)[None, :]
    flat_idx = (valid_starts[:, None] + inner).reshape(-1)
    n_per_batch = valid.sum(axis=1)
    cu_comp = np.zeros(len(cu), dtype=np.int32)
    cu_comp[1:] = np.cumsum(n_per_batch)
    return flat_idx, cu_comp


def _numpy_fallback(k, cu_seqlens):
    flat_idx, cu_comp = _calc_chunks_with_stride(
        np.asarray(cu_seqlens), KERNEL_SIZE, KERNEL_STRIDE)
    k = np.asarray(k)
    gathered = k[flat_idx].reshape(-1, KERNEL_SIZE, k.shape[1], k.shape[2])
    return gathered.mean(axis=1, dtype=np.float64).astype(k.dtype), cu_comp


def _build_w2() -> np.ndarray:
    w2 = np.zeros((128, 128), np.float32)
    idx = np.arange(128)
    w2[idx, idx] = 1.0 / KERNEL_SIZE
    w2[idx[1:], idx[:-1]] = 1.0 / KERNEL_SIZE     # w2[m+1, m]
    return w2


def _prep_shard(x: np.ndarray):
    """x: (ROWS_PER_CORE, F) fp32 -> (kb (TILES,128,16*F), bnd (TILES, F)).
    bnd[i] = block sum of the 16 rows just past tile i (host-applied)."""
    kb = x[:TILES * TILE_ROWS].reshape(TILES, 128, JPB, F) \
        .reshape(TILES, 128, JPB * F)
    bnd = np.stack([
        x[TILE_ROWS * (i + 1): TILE_ROWS * (i + 1) + JPB].sum(
            axis=0, dtype=np.float32)
        for i in range(TILES)])
    return np.ascontiguousarray(kb), bnd


def _run_hw(k2: np.ndarray, trace: bool = False, **spmd_kwargs):
    """k2: (BATCH*SEQ_LEN, F) fp32 contiguous. Returns (per-core outs, results)."""
    from concourse.bass_utils import run_bass_kernel_spmd

    nc = _get_module()
    w2 = _CACHE.setdefault("w2", _build_w2())
    in_maps = []
    bnds = []
    for s in range(BATCH):
        for r0 in (s * SEQ_LEN, s * SEQ_LEN + SEQ_LEN - ROWS_PER_CORE):
            kb, bnd = _prep_shard(k2[r0: r0 + ROWS_PER_CORE])
            in_maps.append({"kb": kb, "w2": w2})
            bnds.append(bnd)
    res = run_bass_kernel_spmd(nc, in_maps, core_ids=list(range(N_CORES)),
                               trace=trace, **spmd_kwargs)
    outs = []
    for i in range(N_CORES):
        o = np.array(res.results[i]["out"])
        # host boundary correction: chunk 127 of tile t also averages the
        # first block of tile t+1 (or the 16 rows past the last tile).
        o[127::128, :] += bnds[i] * (1.0 / KERNEL_SIZE)
        outs.append(o)
    return outs, res


def _assemble(outs) -> np.ndarray:
    seqs = []
    for s in range(BATCH):
        a = outs[2 * s]          # chunks 0..511
        b = outs[2 * s + 1]      # chunks 511..1022 (first is dup of a[511])
        seqs.append(np.concatenate([a, b[1:]], axis=0))
    comp = np.concatenate(seqs, axis=0)
    return np.ascontiguousarray(comp.reshape(-1, HEAD_NUM_K, HEAD_DIM))


def kernel(k, cu_seqlens):
    k = np.asarray(k)
    cu_seqlens = np.asarray(cu_seqlens)
    expected_cu = np.arange(BATCH + 1, dtype=np.int64) * SEQ_LEN
    if (k.shape != (BATCH * SEQ_LEN, HEAD_NUM_K, HEAD_DIM)
            or k.dtype != np.float32
            or cu_seqlens.shape != (BATCH + 1,)
            or not np.array_equal(np.asarray(cu_seqlens, np.int64), expected_cu)):
        return _numpy_fallback(k, cu_seqlens)

    _, cu_comp = _calc_chunks_with_stride(cu_seqlens, KERNEL_SIZE, KERNEL_STRIDE)
    k2 = np.ascontiguousarray(k.reshape(BATCH * SEQ_LEN, F))
    outs, _ = _run_hw(k2)
    return _assemble(outs), cu_comp


# revision 17
# speedup vs baseline: 1.9276x; 1.0742x over previous
"""Trainium2 Bass kernel for nn_CompressK (segment_reduce).

Computes, per sequence, a mean over sliding windows of KERNEL_SIZE=32 rows
at stride KERNEL_STRIDE=16 of k (viewed as (rows, head_num_k*head_dim)),
returning (compressed_k, cu_comp) exactly like the reference.

Hardware strategy (8 NeuronCores):
  - 4 sequences of 16384 rows -> 1023 chunks each. Two cores per sequence:
    core 2s   computes chunks   0..511 (rows [0,     8208) of seq s)
    core 2s+1 computes chunks 511..1022 (rows [8176, 16384) of seq s)
    Both produce 512 chunks; the duplicated chunk 511 is dropped on gather.
  - Sliding mean = block sums + adjacent add: with 16-row blocks,
    out[c] = (B[c] + B[c+1]) / 32. Host pre-blocks the shard so block p of
    tile i sits on SBUF partition p: kb[i, p, j*F+f] = row (2048i+128p+j).
  - Per tile the VectorE accumulates the 16 rows with in-place fp32 adds
    (exact, no PE weight-load serialization, no PSUM); the B[c+1] term
    comes from an SBUF->SBUF partition-shift DMA, with the cross-tile
    boundary block sums computed on host (tiny) to avoid serialization.
    ScalarE applies the 1/32 scale on the way to the output tile.
"""

import numpy as np

KERNEL_SIZE = 32
KERNEL_STRIDE = 16
HEAD_NUM_K = 4
HEAD_DIM = 128
BATCH = 4
SEQ_LEN = 16384
F = HEAD_NUM_K * HEAD_DIM          # 512 features per row
N_CORES = 8
CHUNKS_PER_SEQ = (SEQ_LEN - KERNEL_SIZE) // KERNEL_STRIDE + 1  # 1023
CHUNKS_PER_CORE = 512
TILES = 4                          # 128-block tiles per core
TILE_ROWS = 128 * KERNEL_STRIDE    # 2048 rows per tile
ROWS_PER_CORE = TILES * TILE_ROWS + KERNEL_STRIDE  # 8208
JPB = KERNEL_STRIDE                # rows per block (16)

_CACHE = {}


def _build_module():
    import concourse.tile as tile
    from concourse import bacc, mybir

    nc = bacc.Bacc("TRN2", target_bir_lowering=False, debug=False)
    f32 = mybir.dt.float32
    # kb[i, p, j*F+f] = k_shard[2048*i + 128*p + j*? ...] -- host blocks rows
    # so partition p of tile i holds block (128i+p)'s 16 rows contiguously.
    kb = nc.dram_tensor("kb", [TILES, 128, JPB * F], f32,
                        kind="ExternalInput").ap()
    # w2[p, m] = 1/32 iff p in (m, m+1): adjacent-block add + mean scale as
    # a single small fp32 matmul per tile (out[m,f] = sum_p w2[p,m] blk[p,f]).
    # The cross-tile boundary term (chunk 127 of each tile needs the next
    # tile's first block) is added on HOST -- it already computes those 16-row
    # sums, and the correction touches only 4 output rows per core.
    w2 = nc.dram_tensor("w2", [128, 128], f32, kind="ExternalInput").ap()
    out = nc.dram_tensor("out", [CHUNKS_PER_CORE, F], f32,
                         kind="ExternalOutput").ap()
    HALF = JPB * F // 2            # 4096 floats: rows j=0..7

    with tile.TileContext(nc) as tc:
        with tc.tile_pool(name="data", bufs=3) as dpool, \
             tc.tile_pool(name="wp", bufs=1) as wpool, \
             tc.tile_pool(name="psum", bufs=2, space="PSUM") as ppool, \
             tc.tile_pool(name="outp", bufs=2) as opool:
            w2sb = wpool.tile([128, 128], f32)
            nc.sync.dma_start(w2sb[:], w2[:])

            def tree_sum(d, base, nrows):
                """In-place binary-tree sum of `nrows` row-slices of F floats
                starting at column `base`; result lands at d[:, base:base+F].
                Wide adds first: (58+FD)-cycle DVE cost amortizes overhead."""
                w = nrows // 2
                while w >= 1:
                    nc.vector.tensor_add(d[:, base: base + w * F],
                                         d[:, base: base + w * F],
                                         d[:, base + w * F: base + 2 * w * F])
                    w //= 2

            for i in range(TILES):
                d = dpool.tile([128, JPB * F], f32, tag="d", name=f"d{i}")
                ps = ppool.tile([128, F], f32, tag="ps", name=f"ps{i}")
                # Last tile streams in quarters so the work left after the
                # final DMA byte lands is minimal (pipeline tail).
                nparts = 4 if i == TILES - 1 else 2
                rows_pp = JPB // nparts
                for h in range(nparts):
                    base = rows_pp * F * h
                    nc.sync.dma_start(d[:, base: base + rows_pp * F],
                                      kb[i, :, base: base + rows_pp * F])
                for h in range(nparts):
                    base = rows_pp * F * h
                    tree_sum(d, base, rows_pp)
                    # Partial block sums go straight to PSUM accumulation:
                    # out[m] = sum_p w2[p, m] * (sum over this part's rows).
                    nc.tensor.matmul(ps[:], lhsT=w2sb[:],
                                     rhs=d[:, base: base + F],
                                     start=(h == 0), stop=(h == nparts - 1))
                ot = opool.tile([128, F], f32, tag="ot", name=f"ot{i}")
                nc.scalar.copy(ot[:], ps[:])
                nc.scalar.dma_start(out[128 * i: 128 * (i + 1), :], ot[:])
    nc.compile()
    return nc


def _get_module():
    if "nc" not in _CACHE:
        _CACHE["nc"] = _build_module()
    return _CACHE["nc"]


def _calc_chunks_with_stride(cu_seqlens_np, chunk_size, stride):
    """Host-side mirror of the reference index computation."""
    cu = np.asarray(cu_seqlens_np, dtype=np.int64)
    batch_sizes = cu[1:] - cu[:-1]
    max_seq_len = int(batch_sizes.max())
    max_chunks = max((max_seq_len - chunk_size) // stride + 1, 0)
    offsets = np.arange(0, max_chunks * stride, stride, dtype=np.int64)
    seq_starts = cu[:-1]
    chunk_start = seq_starts[:, None] + offsets[None, :]
    chunk_end = chunk_start + chunk_size
    valid = chunk_end <= (seq_starts[:, None] + batch_sizes[:, None])
    valid_starts = chunk_start[valid]
    inner = np.arange(chunk_size, dtype=np.int64)[None, :]
    flat_idx = (valid_starts[:, None] + inner).reshape(-1)
    n_per_batch = valid.sum(axis=1)
    cu_comp = np.zeros(len(cu), dtype=np.int32)
    cu_comp[1:] = np.cumsum(n_per_batch)
    return flat_idx, cu_comp


def _numpy_fallback(k, cu_seqlens):
    flat_idx, cu_comp = _calc_chunks_with_stride(
        np.asarray(cu_seqlens), KERNEL_SIZE, KERNEL_STRIDE)
    k = np.asarray(k)
    gathered = k[flat_idx].reshape(-1, KERNEL_SIZE, k.shape[1], k.shape[2])
    return gathered.mean(axis=1, dtype=np.float64).astype(k.dtype), cu_comp


def _build_w2() -> np.ndarray:
    w2 = np.zeros((128, 128), np.float32)
    idx = np.arange(128)
    w2[idx, idx] = 1.0 / KERNEL_SIZE
    w2[idx[1:], idx[:-1]] = 1.0 / KERNEL_SIZE     # w2[m+1, m]
    return w2


def _prep_shard(x: np.ndarray):
    """x: (ROWS_PER_CORE, F) fp32 -> (kb (TILES,128,16*F), bnd (TILES, F)).
    bnd[i] = block sum of the 16 rows just past tile i (host-applied)."""
    kb = x[:TILES * TILE_ROWS].reshape(TILES, 128, JPB, F) \
        .reshape(TILES, 128, JPB * F)
    bnd = np.stack([
        x[TILE_ROWS * (i + 1): TILE_ROWS * (i + 1) + JPB].sum(
            axis=0, dtype=np.float32)
        for i in range(TILES)])
    return np.ascontiguousarray(kb), bnd


def _run_hw(k2: np.ndarray, trace: bool = False, **spmd_kwargs):
    """k2: (BATCH*SEQ_LEN, F) fp32 contiguous. Returns (per-core outs, results)."""
    from concourse.bass_utils import run_bass_kernel_spmd

    nc = _get_module()
    w2 = _CACHE.setdefault("w2", _build_w2())
    in_maps = []
    bnds = []
    for s in range(BATCH):
        for r0 in (s * SEQ_LEN, s * SEQ_LEN + SEQ_LEN - ROWS_PER_CORE):
            kb, bnd = _prep_shard(k2[r0: r0 + ROWS_PER_CORE])
            in_maps.append({"kb": kb, "w2": w2})
            bnds.append(bnd)
    res = run_bass_kernel_spmd(nc, in_maps, core_ids=list(range(N_CORES)),
                               trace=trace, **spmd_kwargs)
    outs = []
    for i in range(N_CORES):
        o = np.array(res.results[i]["out"])
        # host boundary correction: chunk 127 of tile t also averages the
        # first block of tile t+1 (or the 16 rows past the last tile).
        o[127::128, :] += bnds[i] * (1.0 / KERNEL_SIZE)
        outs.append(o)
    return outs, res


def _assemble(outs) -> np.ndarray:
    seqs = []
    for s in range(BATCH):
        a = outs[2 * s]          # chunks 0..511
        b = outs[2 * s + 1]      # chunks 511..1022 (first is dup of a[511])
        seqs.append(np.concatenate([a, b[1:]], axis=0))
    comp = np.concatenate(seqs, axis=0)
    return np.ascontiguousarray(comp.reshape(-1, HEAD_NUM_K, HEAD_DIM))


def kernel(k, cu_seqlens):
    k = np.asarray(k)
    cu_seqlens = np.asarray(cu_seqlens)
    expected_cu = np.arange(BATCH + 1, dtype=np.int64) * SEQ_LEN
    if (k.shape != (BATCH * SEQ_LEN, HEAD_NUM_K, HEAD_DIM)
            or k.dtype != np.float32
            or cu_seqlens.shape != (BATCH + 1,)
            or not np.array_equal(np.asarray(cu_seqlens, np.int64), expected_cu)):
        return _numpy_fallback(k, cu_seqlens)

    _, cu_comp = _calc_chunks_with_stride(cu_seqlens, KERNEL_SIZE, KERNEL_STRIDE)
    k2 = np.ascontiguousarray(k.reshape(BATCH * SEQ_LEN, F))
    outs, _ = _run_hw(k2)
    return _assemble(outs), cu_comp
